# revision 2
# baseline (speedup 1.0000x reference)
"""Multi-head attention (B=2, N=2048, C=1024, H=16, qk-RMSNorm) on 8 TRN2 cores.

Restructured v2 of the baseline kernel. Same sharding (TP over 4 head
groups x DP over batch; host sums the 4 w_proj partials per batch).

Scheduling changes vs baseline:
- x arrives in j-block slices (rotating tiles), v/qk-gen/rmsnorm are
  interleaved per block, so the PE starts ~2us in and never stalls on
  DMA (the baseline lost ~60us to startup + stalls + pstate resets).
- Attention pipeline is 2 deep: iteration idx runs S(cur) || AV(prev),
  then norm(prev2) + proj, so the softmax-denominator chain never
  blocks the PE in program order.
- Projection is w-stationary with transposed output [C, n]: bias folds
  into the per-partition DVE merge (saves 16k bias-matmul columns);
  host transposes.
- RMSNorm: both heads of a 128-channel block share one sqrt/recip/mult
  ([128,512] ops, halves DVE+Act work); sum-of-squares broadcast pairs
  two K=64 matmuls in PE row groups h0/h64.
- Softmax denominator broadcast is ONE K=2 matmul per unit against a
  constant selector (rows 63/64), replacing per-head ones-matmuls +
  wide reciprocals.
- q/k/pt/v are bf16 (same PE rate as fp32r, half SBUF/DMA); the qkv/
  proj GEMMs stay fp32r.

PSUM budget (8 banks x 2KB): tag "s2" [128,512] bufs=3 (S slots, qk/v
accs, proj accs), tag "oas" [128,512] bufs=4 (AV accumulators, rmsnorm
sumsq), tag "pc" [128,512] bufs=1 (denominator broadcast).
"""

import sys

if "/opt/trn_rl_repo" not in sys.path:
    sys.path.insert(0, "/opt/trn_rl_repo")

from contextlib import ExitStack

import numpy as np

import concourse.mybir as mybir
import concourse.tile as tile
from concourse import bacc
from concourse.bass_utils import run_bass_kernel_spmd

F32 = mybir.dt.float32
F32R = mybir.dt.float32r
BF16 = mybir.dt.bfloat16
AF = mybir.ActivationFunctionType

B, N, C, H = 2, 2048, 1024, 16
D = C // H          # 64
EPS = 1e-6
NCORES = 8
GROUPS = 4          # head groups (cores per batch)
HL = H // GROUPS    # heads per core = 4
CL = HL * D         # local channels = 256
SCALE = D ** -0.5   # 0.125

P = 128             # partition dim
KT = C // P         # 8 contraction tiles over C
NQ = 512            # query/token block
HPB = P // D        # heads per 128-channel block = 2
VW = D + 1          # 65: v columns + ones column


def build(n=N, nq=NQ):
    nb = n // P          # key blocks of 128
    nj = n // nq         # token blocks of nq
    kt = KT

    nc = bacc.Bacc("TRN2", target_bir_lowering=False, debug=False,
                   num_devices=NCORES)

    xT_d = nc.dram_tensor("xT", [C, n], F32, kind="ExternalInput").ap()
    wqk_d = nc.dram_tensor("w_qk", [C, 2 * CL], F32, kind="ExternalInput").ap()
    wv_d = nc.dram_tensor("w_v", [C, CL], F32, kind="ExternalInput").ap()
    wpr_d = nc.dram_tensor("w_pr", [CL, C], F32, kind="ExternalInput").ap()
    bqk_d = nc.dram_tensor("b_qk", [P, 4], F32, kind="ExternalInput").ap()
    bv_d = nc.dram_tensor("b_v", [1, CL], F32, kind="ExternalInput").ap()
    bprT_d = nc.dram_tensor("b_prT", [P, C // P], F32, kind="ExternalInput").ap()
    qkw_d = nc.dram_tensor("qkw", [P, 4], F32, kind="ExternalInput").ap()
    outT_d = nc.dram_tensor("outT", [C, n], F32, kind="ExternalOutput").ap()

    with tile.TileContext(nc) as tc, ExitStack() as ctx:
        con = ctx.enter_context(tc.tile_pool(name="con", bufs=1))
        wp = ctx.enter_context(tc.tile_pool(name="wp", bufs=1))
        qk = ctx.enter_context(tc.tile_pool(name="qk", bufs=1))
        vp = ctx.enter_context(tc.tile_pool(name="vp", bufs=1))
        xp = ctx.enter_context(tc.tile_pool(name="xp", bufs=1))
        sqp = ctx.enter_context(tc.tile_pool(name="sqp", bufs=4))
        rp = ctx.enter_context(tc.tile_pool(name="rp", bufs=2))
        ptp = ctx.enter_context(tc.tile_pool(name="ptp", bufs=1))
        atp = ctx.enter_context(tc.tile_pool(name="atp", bufs=1))
        rp2 = ctx.enter_context(tc.tile_pool(name="rp2", bufs=2))
        osp = ctx.enter_context(tc.tile_pool(name="osp", bufs=4))
        ps = ctx.enter_context(tc.tile_pool(name="ps", bufs=1, space="PSUM"))

        # ---- small constant DMAs first (tiny; needed by early merges) ----
        bv_sb = con.tile([1, CL], F32R, tag="bv")
        nc.sync.dma_start(bv_sb[:], bv_d[:].bitcast(F32R))
        bqk_sb = con.tile([P, 4], F32, tag="bqk")
        nc.sync.dma_start(bqk_sb[:], bqk_d[:])
        qkw_sb = con.tile([P, 4], F32, tag="qkw")
        nc.sync.dma_start(qkw_sb[:], qkw_d[:])
        bprT_sb = con.tile([P, C // P], F32, tag="bprT")
        nc.sync.dma_start(bprT_sb[:], bprT_d[:])

        # ---- constants (compute engines; no DMA) ----
        ones_f = con.tile([P, P], F32, tag="onesf")
        nc.vector.memset(ones_f[:], 1.0)
        ones_r = con.tile([1, P], F32R, tag="onesr")     # lhsT for v bias bcast
        nc.vector.tensor_copy(ones_r[:], ones_f[0:1, :])
        ones_m = con.tile([P, P], BF16, tag="onesm")     # lhsT for sumsq bcast
        nc.vector.memset(ones_m[:], 1.0)
        eps_sb = con.tile([P, 1], F32, tag="eps")
        nc.vector.memset(eps_sb[:], EPS)
        # ---- weight tiles (DMAs interleaved with x below) ----
        wv_sb = [wp.tile([P, CL], F32R, tag=f"wv{k}", name=f"wv{k}") for k in range(kt)]
        wqk_sb = [wp.tile([P, 2 * CL], F32R, tag=f"wqk{k}", name=f"wqk{k}") for k in range(kt)]
        wpr_sb = [wp.tile([P, C], F32R, tag=f"wpr{t}", name=f"wpr{t}") for t in range(CL // P)]

        # rotating x tiles: 2 j-blocks in flight per k
        def x_tile(j, k):
            return xp.tile([P, nq], F32R, tag=f"xt{k}", bufs=2, name=f"xt{k}_{j}")

        # persistent attention operands
        qkT = [qk.tile([P, n], BF16, tag=f"qkT{m}", name=f"qkT{m}") for m in range(4)]
        v_aug = [vp.tile([P, HL * VW], BF16, tag=f"va{i}", name=f"va{i}") for i in range(nb)]
        attnT = [atp.tile([P, n], F32R, tag=f"at{t}", name=f"at{t}") for t in range(HL // HPB)]

        # startup: v weights + x(j=0) interleaved, then everything else.
        xs = {}
        for k in range(kt):
            nc.sync.dma_start(wv_sb[k][:], wv_d[k * P:(k + 1) * P, :].bitcast(F32R))
            t = x_tile(0, k)
            nc.sync.dma_start(t[:], xT_d[k * P:(k + 1) * P, 0:nq].bitcast(F32R))
            xs[(0, k)] = t
        for k in range(kt):
            nc.sync.dma_start(wqk_sb[k][:], wqk_d[k * P:(k + 1) * P, :].bitcast(F32R))
        for t in range(CL // P):
            nc.sync.dma_start(wpr_sb[t][:], wpr_d[t * P:(t + 1) * P, :].bitcast(F32R))

        # ---- stage 1+2, pipelined per token block j ----
        for j in range(nj):
            js = slice(j * nq, (j + 1) * nq)
            if j + 1 < nj:
                for k in range(kt):
                    t = x_tile(j + 1, k)
                    nc.sync.dma_start(
                        t[:], xT_d[k * P:(k + 1) * P,
                                   (j + 1) * nq:(j + 2) * nq].bitcast(F32R))
                    xs[(j + 1, k)] = t

            # v for this block's nq//P key blocks: natural layout + ones col
            for i in range(j * (nq // P), (j + 1) * (nq // P)):
                ioff = i * P - j * nq
                acc = ps.tile([P, CL], F32, tag="s2", bufs=2, name="vacc")
                for k in range(kt):
                    nc.tensor.matmul(
                        acc[:], xs[(j, k)][:, ioff:ioff + P], wv_sb[k][:],
                        start=(k == 0), stop=False)
                nc.tensor.matmul(acc[:], ones_r[0:1, 0:P], bv_sb[:],
                                 start=False, stop=True)
                for h in range(HL):
                    base = h * VW
                    nc.scalar.copy(
                        v_aug[i][:, base:base + D], acc[:, h * D:(h + 1) * D])
                    nc.gpsimd.tensor_copy(
                        v_aug[i][:, base + D:base + VW], ones_f[:, 0:1])

            # qk projection for this block: qkT[m][:, js], m in 0..3
            for m in range(4):
                acc = ps.tile([P, nq], F32, tag="s2", bufs=2, name="qacc")
                for k in range(kt):
                    nc.tensor.matmul(
                        acc[:], wqk_sb[k][:, m * P:(m + 1) * P],
                        xs[(j, k)][:, :], start=(k == 0), stop=(k == kt - 1))
                nc.vector.tensor_scalar_add(
                    qkT[m][:, js], acc[:], bqk_sb[:, m:m + 1])

            # rmsnorm both heads of each block at once (k blocks first so
            # attention's kT dependency clears earliest)
            for m in (2, 3, 0, 1):
                sq = sqp.tile([P, nq], BF16, tag="sq", name="sq")
                nc.vector.tensor_mul(sq[:], qkT[m][:, js], qkT[m][:, js])
                for h2 in range(HPB):
                    pr = slice(h2 * D, (h2 + 1) * D)
                    ssq = ps.tile([P, nq], F32, tag="oas", bufs=4, name="ssq")
                    nc.tensor.matmul(ssq[:], ones_m[pr, :], sq[pr, :],
                                     start=True, stop=True)
                    rms = rp.tile([P, nq], F32, tag="rms", bufs=4, name="rms")
                    nc.scalar.activation(rms[:], ssq[:], AF.Sqrt,
                                         scale=1.0 / D, bias=eps_sb[:, 0:1])
                    rec = rp.tile([P, nq], F32, tag="rec", bufs=4, name="rec")
                    nc.vector.reciprocal_approx_fast(rec[:], rms[:])
                    nc.vector.scalar_tensor_tensor(
                        qkT[m][pr, js], qkT[m][pr, js], qkw_sb[pr, m:m + 1],
                        rec[pr, :], op0=mybir.AluOpType.mult,
                        op1=mybir.AluOpType.mult)

        # ---- attention: S(cur) || AV(prev), then norm(prev2) + proj ----
        units = [(j, hp) for j in range(nj) for hp in range(HL // HPB)]
        BLK = min(4, nb)

        def emit_s(u, i):
            j, hp = u
            js = slice(j * nq, (j + 1) * nq)
            qm, km = hp, 2 + hp
            s2 = ps.tile([P, 2 * nq], F32, tag="s2", bufs=2, name="s2")
            for sub in range(HPB):
                pr = slice(sub * D, (sub + 1) * D)
                nc.tensor.matmul(
                    s2[:, sub * nq:(sub + 1) * nq],
                    qkT[km][pr, i * P:(i + 1) * P], qkT[qm][pr, js],
                    start=True, stop=True)
            pt = ptp.tile([P, 2 * nq], BF16, tag="pt", bufs=20, name="pt")
            nc.scalar.activation(pt[:], s2[:], AF.Exp, scale=SCALE)
            return pt

        def emit_av(u, oas, pts, i):
            j, hp = u
            for sub in range(HPB):
                h = hp * HPB + sub
                nc.tensor.matmul(
                    oas[sub][0:VW, :], v_aug[i][:, h * VW:(h + 1) * VW],
                    pts[i][:, sub * nq:(sub + 1) * nq],
                    start=(i == 0), stop=(i == nb - 1))

        def emit_norm(u, oas):
            # Softmax denominators ride in row 64 of each AV output. Copy
            # them to 1-partition rows, PE-broadcast each to 64 partitions
            # (both broadcasts share one PSUM slot; base-0 outputs only —
            # nonzero output tile positions with K=1 fail the ISA check),
            # reciprocal into SBUF, then normalize into attnT.
            j, hp = u
            js = slice(j * nq, (j + 1) * nq)
            for sub in range(HPB):
                sums = rp2.tile([1, nq], F32R, tag=f"sums{sub}", name="sums")
                nc.vector.tensor_copy(sums[:], oas[sub][D:VW, :])
                bcs = ps.tile([D, nq], F32, tag="s2", bufs=2, name="bcs")
                nc.tensor.matmul(bcs[:], ones_r[0:1, 0:D], sums[:],
                                 start=True, stop=True)
                recn = rp2.tile([D, nq], F32, tag=f"recn{sub}", name="recn")
                nc.vector.reciprocal_approx_fast(recn[:], bcs[:])
                pr = slice(sub * D, (sub + 1) * D)
                nc.vector.tensor_mul(attnT[hp][pr, js], oas[sub][0:D, :],
                                     recn[:])

        def emit_proj(j):
            js = slice(j * nq, (j + 1) * nq)
            for m in range(C // P):
                acc = ps.tile([P, nq], F32, tag="s2", bufs=2, name="pacc")
                for t in range(CL // P):
                    nc.tensor.matmul(
                        acc[:], wpr_sb[t][:, m * P:(m + 1) * P],
                        attnT[t][:, js], start=(t == 0), stop=(t == CL // P - 1))
                ost = osp.tile([P, nq], F32, tag="ost", name="ost")
                nc.vector.tensor_scalar_add(ost[:], acc[:], bprT_sb[:, m:m + 1])
                nc.sync.dma_start(outT_d[m * P:(m + 1) * P, js], ost[:])

        prev = None    # (unit, oas, pts)
        prev2 = None
        for idx in range(len(units) + 2):
            cur = units[idx] if idx < len(units) else None
            # norm/proj of prev2 first: their reads gate the slot reuse of
            # this iteration's AV writes, so they must precede them in
            # scheduler priority (emitting them later deadlocks the
            # in-order PE queue)
            if prev2 is not None:
                emit_norm(prev2[0], prev2[1])
                j2, hp2 = prev2[0]
                if hp2 == HL // HPB - 1:
                    emit_proj(j2)
            oas_prev = None
            if prev is not None:
                oas_prev = [ps.tile([P, nq], F32, tag="oas", bufs=4,
                                    name=f"oa{s_}") for s_ in range(HPB)]
            pts = {}
            for ib in range((nb + BLK - 1) // BLK):
                blk = range(ib * BLK, min((ib + 1) * BLK, nb))
                if cur is not None:
                    for i in blk:
                        pts[i] = emit_s(cur, i)
                if prev is not None:
                    for i in blk:
                        emit_av(prev[0], oas_prev, prev[2], i)
            prev2 = (prev[0], oas_prev) if prev is not None else None
            prev = (cur, None, pts) if cur is not None else None

    nc.compile()
    return nc


_NC_CACHE = {}


def _get_nc(n=N, nq=NQ):
    key = (n, nq)
    if key not in _NC_CACHE:
        _NC_CACHE[key] = build(n, nq)
    return _NC_CACHE[key]


def make_in_maps(x, w_qkv, b_qkv, q_w, k_w, w_proj, b_proj):
    """Shard full inputs into per-core in_maps (host side)."""
    in_maps = []
    for cid in range(NCORES):
        b, g = cid // GROUPS, cid % GROUPS
        c0 = g * CL
        xT = np.ascontiguousarray(x[b].T)
        w_qk = np.ascontiguousarray(
            np.concatenate([w_qkv[:, c0:c0 + CL],
                            w_qkv[:, C + c0:C + c0 + CL]], axis=1))
        w_v = np.ascontiguousarray(w_qkv[:, 2 * C + c0:2 * C + c0 + CL])
        w_pr = np.ascontiguousarray(w_proj[c0:c0 + CL, :])
        b_qk = np.stack([b_qkv[c0 + m * P:c0 + (m + 1) * P] for m in range(2)]
                        + [b_qkv[C + c0 + m * P:C + c0 + (m + 1) * P]
                           for m in range(2)], axis=1)
        b_v = b_qkv[2 * C + c0:2 * C + c0 + CL].reshape(1, CL)
        # host gather sums GROUPS partials per batch; split the bias so it
        # lands exactly once
        b_prT = np.ascontiguousarray((b_proj / GROUPS).reshape(C // P, P).T)
        qkw = np.stack([np.tile(q_w, HPB), np.tile(q_w, HPB),
                        np.tile(k_w, HPB), np.tile(k_w, HPB)], axis=1)
        in_maps.append({
            "xT": xT.astype(np.float32),
            "w_qk": w_qk.astype(np.float32),
            "w_v": w_v.astype(np.float32),
            "w_pr": w_pr.astype(np.float32),
            "b_qk": np.ascontiguousarray(b_qk).astype(np.float32),
            "b_v": b_v.astype(np.float32),
            "b_prT": b_prT.astype(np.float32),
            "qkw": np.ascontiguousarray(qkw).astype(np.float32),
        })
    return in_maps


def kernel(x, w_qkv, b_qkv, q_w, k_w, w_proj, b_proj, _trace=False):
    x = np.asarray(x)
    n = x.shape[1]
    nc = _get_nc(n, NQ if n % NQ == 0 else P)
    in_maps = make_in_maps(np.asarray(x, np.float32), np.asarray(w_qkv, np.float32),
                           np.asarray(b_qkv, np.float32), np.asarray(q_w, np.float32),
                           np.asarray(k_w, np.float32), np.asarray(w_proj, np.float32),
                           np.asarray(b_proj, np.float32))
    res = run_bass_kernel_spmd(nc, in_maps, core_ids=list(range(NCORES)),
                               trace=_trace)
    # TP unshard: sum the 4 head-group partials per batch, transpose, stack
    out = np.stack([
        sum(res.results[b * GROUPS + g]["outT"] for g in range(GROUPS)).T
        for b in range(B)
    ]).astype(np.float32)
    if _trace:
        return out, res
    return out
